# revision 6
# baseline (speedup 1.0000x reference)
"""Cross-attention Trainium2 kernel (B=8, N=2048, C=768, head=1), fp8 edition.

reference:
  q = q_x @ Wq.T ; k = k_x @ Wk.T
  S = (q @ k.T) / 768 ; P = softmax(S, -1) ; out = P @ v_x

Math restructuring (per core, data-parallel over batch):
  M  = Wq.T @ Wk                      (host weight-fold, fp8)
  tT = (q_x @ M).T      [c2, n]       fp8 DoubleRow matmul, qT shipped fp8
  ST[m, n] = k_x @ tT                 fp8 DoubleRow (kT shipped fp8)
  E = exp(ST/768); e = E - 1          logits are tiny (|l| < ~0.3), so the
                                      softmax is near-uniform: E in [0.75,1.35]
  out*Z = colsum(v) + e^T @ [v|1]     mean-subtraction lets the big PV matmul
                                      run in fp8: e has full fp8 relative
                                      precision while colsum(v) (the dominant
                                      term) is computed from bf16 v exactly.
  Z = 2048 + sum_m e[m, n]            (the [v|1] ones-column + colsum fold)
  out[n, c] = (colsum_v[c] + (e^T v)[n, c]) / Z[n]

All three big matmuls run as fp8e4m3 with perf_mode=DoubleRow (2 contraction
rows per PE cell per cycle).  Host-side prep is formatting only: dtype casts,
transposes, padding, plus the weight-only fold M = Wq.T @ Wk.

Engine split: PE matmuls; Scalar exp + final out*(1/Z); GpSimd e=E-1 (fp8
quantize, SBUF-only); DVE tT fp8 casts, colsum-broadcast adds, reciprocal.
"""

import sys

sys.path.insert(0, "/opt/trn_rl_repo")

from contextlib import ExitStack

import numpy as np

import concourse.bass as bass
import concourse.mybir as mybir
import concourse.tile as tile
from concourse import bacc

F32 = mybir.dt.float32
F32R = mybir.dt.float32r
BF16 = mybir.dt.bfloat16
F8 = mybir.dt.float8e4
DR = mybir.MatmulPerfMode.DoubleRow

B = 8
N = 2048
C = 768
P = 128
CC = C // P          # 6 chunks of the channel dims (c1 and c2)
NN = N // P          # 16 chunks of the sequence dim
BLK = 512            # S / tT free-dim block (PSUM bank = 512 f32)
NB = N // BLK        # 4 sequence blocks
VW = 784             # padded v row: [v(768) | 1 1 | 0-pad], stride % 16 == 0
H0 = 384             # PV free-dim split: [0:384] and [384:770]
H1 = 386
ZCOL = C             # denominator column (768) inside the 770-wide PV output
SCALE = 1.0 / float(C)
EXP = mybir.ActivationFunctionType.Exp
COPY = mybir.ActivationFunctionType.Copy


def build_kernel():
    nc = bacc.Bacc("TRN2", target_bir_lowering=False, debug=False, num_devices=B)
    qT = nc.declare_dram_parameter("qT", [C, N], F8, isOutput=False)
    kT = nc.declare_dram_parameter("kT", [C, N], F8, isOutput=False)
    M8 = nc.declare_dram_parameter("M8", [C, C], F8, isOutput=False)
    vbf = nc.declare_dram_parameter("vbf", [N, VW], BF16, isOutput=False)
    vb8 = nc.declare_dram_parameter("vb8", [N, VW], F8, isOutput=False)
    out = nc.declare_dram_parameter("out", [N, C], F32, isOutput=True)

    with tile.TileContext(nc) as tc, ExitStack() as ctx:
        persist = ctx.enter_context(tc.tile_pool(name="persist", bufs=1))
        qTs = persist.tile([P, CC, N], F8)      # q_x.T  [c1, n]
        kTs = persist.tile([P, CC, N], F8)      # k_x.T  [c2, m]
        M8s = persist.tile([P, CC, C], F8)      # M      [c1, c2]
        vbfs = persist.tile([P, NN, VW], BF16)  # [v|1|1|0]  bf16 (for colsum)
        vb8s = persist.tile([P, NN, VW], F8)    # [v|1|1|0]  fp8 (for PV)
        tT8 = persist.tile([P, CC, N], F8)      # (q_x @ M).T  [c2, n]
        T1b = persist.tile([P, VW], F32)        # colsum(v) broadcast to 128 rows
        onesW = persist.tile([P, P], BF16)      # all-ones lhsT for colsum
        nc.vector.memset(onesW, 1.0)

        # ---------------- DMA: critical path first ----------------
        # tT(0) needs qT block 0 + all of M; S(0) then walks kT m-groups.
        for cc in range(CC):
            nc.sync.dma_start(
                out=qTs[:, cc, 0:BLK],
                in_=qT[cc * P : (cc + 1) * P, 0:BLK],
            )
            nc.sync.dma_start(
                out=M8s[:, cc, :], in_=M8[cc * P : (cc + 1) * P, :]
            )
        for g in range(4):
            for cc in range(CC):
                nc.sync.dma_start(
                    out=kTs[:, cc, g * BLK : (g + 1) * BLK],
                    in_=kT[cc * P : (cc + 1) * P, g * BLK : (g + 1) * BLK],
                )
        for mc in range(NN):
            nc.sync.dma_start(
                out=vb8s[:, mc, :], in_=vb8[mc * P : (mc + 1) * P, :]
            )
            nc.sync.dma_start(
                out=vbfs[:, mc, :], in_=vbf[mc * P : (mc + 1) * P, :]
            )
        for g in range(1, 4):
            for cc in range(CC):
                nc.sync.dma_start(
                    out=qTs[:, cc, g * BLK : (g + 1) * BLK],
                    in_=qT[cc * P : (cc + 1) * P, g * BLK : (g + 1) * BLK],
                )

        # ---------------- PE warmup (HAM un-throttle) ----------------
        with (
            tc.tile_pool(name="warm", bufs=1) as warm_pool,
            tc.tile_pool(name="warm_psum", bufs=1, space="PSUM") as warm_psum,
        ):
            wl = warm_pool.tile([P, P], BF16)
            wr = warm_pool.tile([P, BLK], BF16)
            nc.vector.memset(wl, 0.0)
            nc.vector.memset(wr, 0.0)
            wps = warm_psum.tile([P, BLK], F32)
            for i in range(16):
                nc.tensor.matmul(wps, wl, wr, start=True, stop=True)

        t_psum = ctx.enter_context(tc.tile_pool(name="t_psum", bufs=1, space="PSUM"))
        s_psum = ctx.enter_context(tc.tile_pool(name="s_psum", bufs=3, space="PSUM"))

        def tt_block(nb):
            # tT[c2, n-block] = M.T-contraction of qT, fp8 DoubleRow over c1.
            # psum->fp8 cast on Scalar: keeps DVE's FIFO free for the PV-psum
            # releasing adds.
            for c2c in range(CC):
                tp = t_psum.tile([P, BLK], F32, tag="tp", name=f"tp{nb}_{c2c}")
                for j in range(CC // 2):
                    nc.tensor.matmul(
                        tp,
                        M8s[:, 2 * j : 2 * j + 2, c2c * P : (c2c + 1) * P],
                        qTs[:, 2 * j : 2 * j + 2, nb * BLK : (nb + 1) * BLK],
                        start=(j == 0),
                        stop=(j == CC // 2 - 1),
                        perf_mode=DR,
                    )
                nc.scalar.activation(
                    out=tT8[:, c2c, nb * BLK : (nb + 1) * BLK], in_=tp, func=COPY
                )

        tt_block(0)

        # ---------------- steady: S -> exp -> e8 -> PV ----------------
        with (
            tc.tile_pool(name="e8_pool", bufs=2) as e8_pool,
            tc.tile_pool(name="ebf_pool", bufs=4) as ebf_pool,
            tc.tile_pool(name="osb_pool", bufs=2) as osb_pool,
            tc.tile_pool(name="out_pool", bufs=2) as out_pool,
            tc.tile_pool(name="rec_pool", bufs=2) as rec_pool,
            tc.tile_pool(name="o_psum", bufs=2, space="PSUM") as o_psum,
            tc.tile_pool(name="o2_psum", bufs=2, space="PSUM") as o2_psum,
        ):
            for nb in range(NB):
                e8b = e8_pool.tile([P, NN, BLK], F8, tag="e8", name=f"e8_{nb}")
                for mc in range(NN):
                    sp = s_psum.tile([P, BLK], F32, tag="sp", name=f"sp{nb}_{mc}")
                    for j in range(CC // 2):
                        nc.tensor.matmul(
                            sp,
                            kTs[:, 2 * j : 2 * j + 2, mc * P : (mc + 1) * P],
                            tT8[:, 2 * j : 2 * j + 2, nb * BLK : (nb + 1) * BLK],
                            start=(j == 0),
                            stop=(j == CC // 2 - 1),
                            perf_mode=DR,
                        )
                    ebf = ebf_pool.tile([P, BLK], BF16, tag="ebf", name=f"eb{nb}_{mc}")
                    nc.scalar.activation(out=ebf, in_=sp, func=EXP, scale=SCALE)
                    nc.vector.tensor_scalar_add(
                        out=e8b[:, mc, :], in0=ebf, scalar1=-1.0
                    )

                if nb == 0:
                    # colsum(vbf) -> T1b broadcast (ones lhsT replicates the
                    # row-sum to all 128 partitions for free)
                    t1a = o_psum.tile([P, H0], F32, tag="opa", name="t1a")
                    t1c = o2_psum.tile([P, H1], F32, tag="opb", name="t1b")
                    for mc in range(NN):
                        nc.tensor.matmul(
                            t1a, onesW, vbfs[:, mc, 0:H0],
                            start=(mc == 0), stop=(mc == NN - 1),
                        )
                    for mc in range(NN):
                        nc.tensor.matmul(
                            t1c, onesW, vbfs[:, mc, H0 : H0 + H1],
                            start=(mc == 0), stop=(mc == NN - 1),
                        )
                    nc.vector.tensor_copy(out=T1b[:, 0:H0], in_=t1a)
                    nc.vector.tensor_copy(out=T1b[:, H0 : H0 + H1], in_=t1c)

                if nb + 1 < NB:
                    tt_block(nb + 1)

                # PV: out*Z[n-sub, 0:770] = e8^T @ vb8 (fp8 DR), + T1b, / Z
                for ns in range(4):
                    opa = o_psum.tile([P, H0], F32, tag="opa", name=f"oa{nb}_{ns}")
                    opb = o2_psum.tile([P, H1], F32, tag="opb", name=f"ob{nb}_{ns}")
                    for mcp in range(NN // 2):
                        lhs = e8b[:, 2 * mcp : 2 * mcp + 2, ns * P : (ns + 1) * P]
                        nc.tensor.matmul(
                            opa, lhs, vb8s[:, 2 * mcp : 2 * mcp + 2, 0:H0],
                            start=(mcp == 0), stop=(mcp == NN // 2 - 1),
                            perf_mode=DR,
                        )
                    for mcp in range(NN // 2):
                        lhs = e8b[:, 2 * mcp : 2 * mcp + 2, ns * P : (ns + 1) * P]
                        nc.tensor.matmul(
                            opb, lhs, vb8s[:, 2 * mcp : 2 * mcp + 2, H0 : H0 + H1],
                            start=(mcp == 0), stop=(mcp == NN // 2 - 1),
                            perf_mode=DR,
                        )
                    o_sb = osb_pool.tile([P, VW], F32, tag="osb", name=f"os{nb}_{ns}")
                    nc.vector.tensor_add(out=o_sb[:, 0:H0], in0=opa, in1=T1b[:, 0:H0])
                    nc.vector.tensor_add(
                        out=o_sb[:, H0 : H0 + H1], in0=opb, in1=T1b[:, H0 : H0 + H1]
                    )
                    rec = rec_pool.tile([P, 1], F32, tag="rec", name=f"rc{nb}_{ns}")
                    nc.vector.reciprocal(out=rec, in_=o_sb[:, ZCOL : ZCOL + 1])
                    o_t = out_pool.tile([P, C], F32, tag="ot", name=f"ot{nb}_{ns}")
                    nc.scalar.activation(
                        out=o_t, in_=o_sb[:, 0:C], func=COPY, scale=rec
                    )
                    row0 = nb * BLK + ns * P
                    nc.sync.dma_start(out=out[row0 : row0 + P, :], in_=o_t)

    nc.compile()
    return nc


_NC = None


def _get_nc():
    global _NC
    if _NC is None:
        _NC = build_kernel()
    return _NC


F8NP = mybir.dt.np(F8)
BF16NP = mybir.dt.np(BF16)


def host_prepare(q_x, k_x, v_x, Wq, Wk):
    """Formatting-only host prep: dtype casts, transposes, padding, plus the
    weight-only fold M = Wq.T @ Wk."""
    q_x = np.ascontiguousarray(np.asarray(q_x, dtype=np.float32))
    k_x = np.ascontiguousarray(np.asarray(k_x, dtype=np.float32))
    v_x = np.ascontiguousarray(np.asarray(v_x, dtype=np.float32))
    Wq = np.asarray(Wq, dtype=np.float32)
    Wk = np.asarray(Wk, dtype=np.float32)
    M8np = np.ascontiguousarray((Wq.T @ Wk).astype(F8NP))
    in_maps = []
    for b in range(B):
        qTb = np.ascontiguousarray(q_x[b].astype(F8NP).T)
        kTb = np.ascontiguousarray(k_x[b].astype(F8NP).T)
        vpad = np.zeros((N, VW), np.float32)
        vpad[:, :C] = v_x[b]
        vpad[:, C : C + 2] = 1.0
        in_maps.append(
            {
                "qT": qTb,
                "kT": kTb,
                "M8": M8np,
                "vbf": vpad.astype(BF16NP),
                "vb8": vpad.astype(F8NP),
            }
        )
    return in_maps


def kernel(q_x, k_x, v_x, Wq, Wk):
    from concourse.bass_utils import run_bass_kernel_spmd

    nc = _get_nc()
    in_maps = host_prepare(q_x, k_x, v_x, Wq, Wk)
    res = run_bass_kernel_spmd(nc, in_maps, core_ids=list(range(B)))
    return np.stack([res.results[i]["out"] for i in range(B)], axis=0)


# revision 9
# speedup vs baseline: 1.2011x; 1.2011x over previous
"""Cross-attention Trainium2 kernel (B=8, N=2048, C=768, head=1), fp8 edition.

reference:
  q = q_x @ Wq.T ; k = k_x @ Wk.T
  S = (q @ k.T) / 768 ; P = softmax(S, -1) ; out = P @ v_x

Math restructuring (per core, data-parallel over batch):
  M  = Wq.T @ Wk                      (host weight-fold, fp8)
  tT = (q_x @ M).T      [c2, n]       fp8 DoubleRow matmul, qT shipped fp8
  ST[m, n] = k_x @ tT                 fp8 DoubleRow (kT shipped fp8)
  E = exp(ST/768); e = E - 1          logits are tiny (|l| < ~0.3), so the
                                      softmax is near-uniform: E in [0.75,1.35]
  out*Z = colsum(v) + e^T @ [v|1]     mean-subtraction lets the big PV matmul
                                      run in fp8: e has full fp8 relative
                                      precision while colsum(v) (the dominant
                                      term) is computed from bf16 v exactly.
  Z = 2048 + sum_m e[m, n]            (the [v|1] ones-column + colsum fold)
  out[n, c] = (colsum_v[c] + (e^T v)[n, c]) / Z[n]

All three big matmuls run as fp8e4m3 with perf_mode=DoubleRow (2 contraction
rows per PE cell per cycle).  Host-side prep is formatting only: dtype casts,
transposes, padding, plus the weight-only fold M = Wq.T @ Wk.

Engine split: PE matmuls; Scalar exp + final out*(1/Z); GpSimd e=E-1 (fp8
quantize, SBUF-only); DVE tT fp8 casts, colsum-broadcast adds, reciprocal.
"""

import sys

sys.path.insert(0, "/opt/trn_rl_repo")

from contextlib import ExitStack

import numpy as np

import concourse.bass as bass
import concourse.mybir as mybir
import concourse.tile as tile
from concourse import bacc

F32 = mybir.dt.float32
F32R = mybir.dt.float32r
BF16 = mybir.dt.bfloat16
F8 = mybir.dt.float8e4
DR = mybir.MatmulPerfMode.DoubleRow

B = 8
N = 2048
C = 768
P = 128
CC = C // P          # 6 chunks of the channel dims (c1 and c2)
NN = N // P          # 16 chunks of the sequence dim
BLK = 512            # S / tT free-dim block (PSUM bank = 512 f32)
NB = N // BLK        # 4 sequence blocks
VW = 784             # padded v row: [v(768) | 1 1 | 0-pad], stride % 16 == 0
H0 = 384             # PV free-dim split: [0:384] and [384:770]
H1 = 386
ZCOL = C             # denominator column (768) inside the 770-wide PV output
SCALE = 1.0 / float(C)
EXP = mybir.ActivationFunctionType.Exp
COPY = mybir.ActivationFunctionType.Copy


def build_kernel():
    nc = bacc.Bacc("TRN2", target_bir_lowering=False, debug=False, num_devices=B)
    qT = nc.declare_dram_parameter("qT", [C, N], F8, isOutput=False)
    kT = nc.declare_dram_parameter("kT", [C, N], F8, isOutput=False)
    M8 = nc.declare_dram_parameter("M8", [C, C], F8, isOutput=False)
    vbf = nc.declare_dram_parameter("vbf", [N, VW], BF16, isOutput=False)
    vb8 = nc.declare_dram_parameter("vb8", [N, VW], F8, isOutput=False)
    out = nc.declare_dram_parameter("out", [N, C], F32, isOutput=True)

    with tile.TileContext(nc) as tc, ExitStack() as ctx:
        persist = ctx.enter_context(tc.tile_pool(name="persist", bufs=1))
        qTs = persist.tile([P, CC, N], F8)      # q_x.T  [c1, n]
        kTs = persist.tile([P, CC, N], F8)      # k_x.T  [c2, m]
        M8s = persist.tile([P, CC, C], F8)      # M      [c1, c2]
        vbfs = persist.tile([P, NN, VW], BF16)  # [v|1|1|0]  bf16 (for colsum)
        vb8s = persist.tile([P, NN, VW], F8)    # [v|1|1|0]  fp8 (for PV)
        tT8 = persist.tile([P, CC, N], F8)      # (q_x @ M).T  [c2, n]
        T1b = persist.tile([P, VW], F32)        # colsum(v) broadcast to 128 rows
        onesW = persist.tile([P, P], BF16)      # all-ones lhsT for colsum
        nc.vector.memset(onesW, 1.0)

        # ---------------- DMA: critical path first ----------------
        # tT(0) needs qT block 0 + all of M; S(0) then walks kT m-groups.
        for cc in range(CC):
            nc.sync.dma_start(
                out=qTs[:, cc, 0:BLK],
                in_=qT[cc * P : (cc + 1) * P, 0:BLK],
            )
            nc.sync.dma_start(
                out=M8s[:, cc, :], in_=M8[cc * P : (cc + 1) * P, :]
            )
        for g in range(4):
            for cc in range(CC):
                nc.sync.dma_start(
                    out=kTs[:, cc, g * BLK : (g + 1) * BLK],
                    in_=kT[cc * P : (cc + 1) * P, g * BLK : (g + 1) * BLK],
                )
        for mc in range(NN):
            nc.sync.dma_start(
                out=vb8s[:, mc, :], in_=vb8[mc * P : (mc + 1) * P, :]
            )
            nc.sync.dma_start(
                out=vbfs[:, mc, :], in_=vbf[mc * P : (mc + 1) * P, :]
            )
        for g in range(1, 4):
            for cc in range(CC):
                nc.sync.dma_start(
                    out=qTs[:, cc, g * BLK : (g + 1) * BLK],
                    in_=qT[cc * P : (cc + 1) * P, g * BLK : (g + 1) * BLK],
                )

        # ---------------- PE warmup (HAM un-throttle) ----------------
        with (
            tc.tile_pool(name="warm", bufs=1) as warm_pool,
            tc.tile_pool(name="warm_psum", bufs=1, space="PSUM") as warm_psum,
        ):
            wl = warm_pool.tile([P, P], BF16)
            wr = warm_pool.tile([P, BLK], BF16)
            nc.vector.memset(wl, 0.0)
            nc.vector.memset(wr, 0.0)
            wps = warm_psum.tile([P, BLK], F32)
            for i in range(16):
                nc.tensor.matmul(wps, wl, wr, start=True, stop=True)

        t_psum = ctx.enter_context(tc.tile_pool(name="t_psum", bufs=2, space="PSUM"))
        s_psum = ctx.enter_context(tc.tile_pool(name="s_psum", bufs=2, space="PSUM"))

        def tt_block(nb):
            # tT[c2, n-block] = M.T-contraction of qT, fp8 DoubleRow over c1
            for c2c in range(CC):
                tp = t_psum.tile([P, BLK], F32, tag="tp", name=f"tp{nb}_{c2c}")
                for j in range(CC // 2):
                    nc.tensor.matmul(
                        tp,
                        M8s[:, 2 * j : 2 * j + 2, c2c * P : (c2c + 1) * P],
                        qTs[:, 2 * j : 2 * j + 2, nb * BLK : (nb + 1) * BLK],
                        start=(j == 0),
                        stop=(j == CC // 2 - 1),
                        perf_mode=DR,
                    )
                nc.vector.tensor_copy(
                    out=tT8[:, c2c, nb * BLK : (nb + 1) * BLK], in_=tp
                )

        tt_block(0)

        # ---------------- steady: S -> exp -> e8 -> PV ----------------
        with (
            tc.tile_pool(name="e8_pool", bufs=2) as e8_pool,
            tc.tile_pool(name="ebf_pool", bufs=4) as ebf_pool,
            tc.tile_pool(name="osb_pool", bufs=2) as osb_pool,
            tc.tile_pool(name="out_pool", bufs=2) as out_pool,
            tc.tile_pool(name="rec_pool", bufs=2) as rec_pool,
            tc.tile_pool(name="o_psum", bufs=2, space="PSUM") as o_psum,
            tc.tile_pool(name="o2_psum", bufs=2, space="PSUM") as o2_psum,
        ):
            for nb in range(NB):
                e8b = e8_pool.tile([P, NN, BLK], F8, tag="e8", name=f"e8_{nb}")
                for mc in range(NN):
                    sp = s_psum.tile([P, BLK], F32, tag="sp", name=f"sp{nb}_{mc}")
                    for j in range(CC // 2):
                        nc.tensor.matmul(
                            sp,
                            kTs[:, 2 * j : 2 * j + 2, mc * P : (mc + 1) * P],
                            tT8[:, 2 * j : 2 * j + 2, nb * BLK : (nb + 1) * BLK],
                            start=(j == 0),
                            stop=(j == CC // 2 - 1),
                            perf_mode=DR,
                        )
                    ebf = ebf_pool.tile([P, BLK], BF16, tag="ebf", name=f"eb{nb}_{mc}")
                    nc.scalar.activation(out=ebf, in_=sp, func=EXP, scale=SCALE)
                    nc.vector.tensor_scalar_add(
                        out=e8b[:, mc, :], in0=ebf, scalar1=-1.0
                    )

                if nb == 0:
                    # colsum(vbf) -> T1b broadcast (ones lhsT replicates the
                    # row-sum to all 128 partitions for free)
                    t1a = o_psum.tile([P, H0], F32, tag="opa", name="t1a")
                    t1c = o2_psum.tile([P, H1], F32, tag="opb", name="t1b")
                    for mc in range(NN):
                        nc.tensor.matmul(
                            t1a, onesW, vbfs[:, mc, 0:H0],
                            start=(mc == 0), stop=(mc == NN - 1),
                        )
                    for mc in range(NN):
                        nc.tensor.matmul(
                            t1c, onesW, vbfs[:, mc, H0 : H0 + H1],
                            start=(mc == 0), stop=(mc == NN - 1),
                        )
                    nc.vector.tensor_copy(out=T1b[:, 0:H0], in_=t1a)
                    nc.vector.tensor_copy(out=T1b[:, H0 : H0 + H1], in_=t1c)

                # PV: out*Z[n-sub, 0:770] = e8^T @ vb8 (fp8 DR), + T1b, / Z
                # (tT(nb+1) is emitted AFTER PV so the tT psum->fp8 casts queue
                # behind the PV-psum-releasing adds in the DVE FIFO, not ahead)
                for ns in range(4):
                    opa = o_psum.tile([P, H0], F32, tag="opa", name=f"oa{nb}_{ns}")
                    opb = o2_psum.tile([P, H1], F32, tag="opb", name=f"ob{nb}_{ns}")
                    for mcp in range(NN // 2):
                        lhs = e8b[:, 2 * mcp : 2 * mcp + 2, ns * P : (ns + 1) * P]
                        nc.tensor.matmul(
                            opa, lhs, vb8s[:, 2 * mcp : 2 * mcp + 2, 0:H0],
                            start=(mcp == 0), stop=(mcp == NN // 2 - 1),
                            perf_mode=DR,
                        )
                    for mcp in range(NN // 2):
                        lhs = e8b[:, 2 * mcp : 2 * mcp + 2, ns * P : (ns + 1) * P]
                        nc.tensor.matmul(
                            opb, lhs, vb8s[:, 2 * mcp : 2 * mcp + 2, H0 : H0 + H1],
                            start=(mcp == 0), stop=(mcp == NN // 2 - 1),
                            perf_mode=DR,
                        )
                    o_sb = osb_pool.tile([P, VW], F32, tag="osb", name=f"os{nb}_{ns}")
                    nc.vector.tensor_add(out=o_sb[:, 0:H0], in0=opa, in1=T1b[:, 0:H0])
                    nc.vector.tensor_add(
                        out=o_sb[:, H0 : H0 + H1], in0=opb, in1=T1b[:, H0 : H0 + H1]
                    )
                    rec = rec_pool.tile([P, 1], F32, tag="rec", name=f"rc{nb}_{ns}")
                    nc.vector.reciprocal(out=rec, in_=o_sb[:, ZCOL : ZCOL + 1])
                    o_t = out_pool.tile([P, C], F32, tag="ot", name=f"ot{nb}_{ns}")
                    nc.scalar.activation(
                        out=o_t, in_=o_sb[:, 0:C], func=COPY, scale=rec
                    )
                    row0 = nb * BLK + ns * P
                    nc.sync.dma_start(out=out[row0 : row0 + P, :], in_=o_t)

                if nb + 1 < NB:
                    tt_block(nb + 1)

    nc.compile()
    return nc


_NC = None


def _get_nc():
    global _NC
    if _NC is None:
        _NC = build_kernel()
    return _NC


F8NP = mybir.dt.np(F8)
BF16NP = mybir.dt.np(BF16)


def host_prepare(q_x, k_x, v_x, Wq, Wk):
    """Formatting-only host prep: dtype casts, transposes, padding, plus the
    weight-only fold M = Wq.T @ Wk."""
    q_x = np.ascontiguousarray(np.asarray(q_x, dtype=np.float32))
    k_x = np.ascontiguousarray(np.asarray(k_x, dtype=np.float32))
    v_x = np.ascontiguousarray(np.asarray(v_x, dtype=np.float32))
    Wq = np.asarray(Wq, dtype=np.float32)
    Wk = np.asarray(Wk, dtype=np.float32)
    M8np = np.ascontiguousarray((Wq.T @ Wk).astype(F8NP))
    in_maps = []
    for b in range(B):
        qTb = np.ascontiguousarray(q_x[b].astype(F8NP).T)
        kTb = np.ascontiguousarray(k_x[b].astype(F8NP).T)
        vpad = np.zeros((N, VW), np.float32)
        vpad[:, :C] = v_x[b]
        vpad[:, C : C + 2] = 1.0
        in_maps.append(
            {
                "qT": qTb,
                "kT": kTb,
                "M8": M8np,
                "vbf": vpad.astype(BF16NP),
                "vb8": vpad.astype(F8NP),
            }
        )
    return in_maps


def kernel(q_x, k_x, v_x, Wq, Wk):
    from concourse.bass_utils import run_bass_kernel_spmd

    nc = _get_nc()
    in_maps = host_prepare(q_x, k_x, v_x, Wq, Wk)
    res = run_bass_kernel_spmd(nc, in_maps, core_ids=list(range(B)))
    return np.stack([res.results[i]["out"] for i in range(B)], axis=0)


# revision 12
# speedup vs baseline: 1.2183x; 1.0143x over previous
"""Cross-attention Trainium2 kernel (B=8, N=2048, C=768, head=1), fp8 edition.

reference:
  q = q_x @ Wq.T ; k = k_x @ Wk.T
  S = (q @ k.T) / 768 ; P = softmax(S, -1) ; out = P @ v_x

Math restructuring (per core, data-parallel over batch):
  M  = Wq.T @ Wk                      (host weight-fold, fp8)
  tT = (q_x @ M).T      [c2, n]       fp8 DoubleRow matmul, qT shipped fp8
  ST[m, n] = k_x @ tT                 fp8 DoubleRow (kT shipped fp8)
  E = exp(ST/768); e = E - 1          logits are tiny (|l| < ~0.3), so the
                                      softmax is near-uniform: E in [0.75,1.35]
  out*Z = colsum(v) + e^T @ [v|1]     mean-subtraction lets the big PV matmul
                                      run in fp8: e has full fp8 relative
                                      precision while colsum(v) (the dominant
                                      term) is computed from bf16 v exactly.
  Z = 2048 + sum_m e[m, n]            (the [v|1] ones-column + colsum fold)
  out[n, c] = (colsum_v[c] + (e^T v)[n, c]) / Z[n]

All three big matmuls run as fp8e4m3 with perf_mode=DoubleRow (2 contraction
rows per PE cell per cycle).  Host-side prep is formatting only: dtype casts,
transposes, padding, plus the weight-only fold M = Wq.T @ Wk.

Engine split: PE matmuls; Scalar exp + final out*(1/Z); GpSimd e=E-1 (fp8
quantize, SBUF-only); DVE tT fp8 casts, colsum-broadcast adds, reciprocal.
"""

import sys

sys.path.insert(0, "/opt/trn_rl_repo")

from contextlib import ExitStack

import numpy as np

import concourse.bass as bass
import concourse.mybir as mybir
import concourse.tile as tile
from concourse import bacc

F32 = mybir.dt.float32
F32R = mybir.dt.float32r
BF16 = mybir.dt.bfloat16
F8 = mybir.dt.float8e4
DR = mybir.MatmulPerfMode.DoubleRow

B = 8
N = 2048
C = 768
P = 128
CC = C // P          # 6 chunks of the channel dims (c1 and c2)
NN = N // P          # 16 chunks of the sequence dim
BLK = 512            # S / tT free-dim block (PSUM bank = 512 f32)
NB = N // BLK        # 4 sequence blocks
VW = 784             # padded v row: [v(768) | 1 1 | 0-pad], stride % 16 == 0
H0 = 384             # PV free-dim split: [0:384] and [384:770]
H1 = 386
ZCOL = C             # denominator column (768) inside the 770-wide PV output
SCALE = 1.0 / float(C)
EXP = mybir.ActivationFunctionType.Exp
COPY = mybir.ActivationFunctionType.Copy


def build_kernel():
    nc = bacc.Bacc("TRN2", target_bir_lowering=False, debug=False, num_devices=B)
    qT = nc.declare_dram_parameter("qT", [C, N], F8, isOutput=False)
    kT = nc.declare_dram_parameter("kT", [C, N], F8, isOutput=False)
    M8 = nc.declare_dram_parameter("M8", [C, C], F8, isOutput=False)
    vbf = nc.declare_dram_parameter("vbf", [N, VW], BF16, isOutput=False)
    vb8 = nc.declare_dram_parameter("vb8", [N, VW], F8, isOutput=False)
    out = nc.declare_dram_parameter("out", [N, C], F32, isOutput=True)

    with tile.TileContext(nc) as tc, ExitStack() as ctx:
        persist = ctx.enter_context(tc.tile_pool(name="persist", bufs=1))
        qTs = persist.tile([P, CC, N], F8)      # q_x.T  [c1, n]
        kTs = persist.tile([P, CC, N], F8)      # k_x.T  [c2, m]
        M8s = persist.tile([P, CC, C], F8)      # M      [c1, c2]
        vbfs = persist.tile([P, NN, VW], BF16)  # [v|1|1|0]  bf16 (for colsum)
        vb8s = persist.tile([P, NN, VW], F8)    # [v|1|1|0]  fp8 (for PV)
        tT8 = persist.tile([P, CC, N], F8)      # (q_x @ M).T  [c2, n]
        T1b = persist.tile([P, VW], F32)        # colsum(v) broadcast to 128 rows
        onesW = persist.tile([P, P], BF16)      # all-ones lhsT for colsum
        nc.vector.memset(onesW, 1.0)

        # ---------------- DMA: critical path first ----------------
        # tT(0) needs qT block 0 + all of M; S(0) then walks kT m-groups.
        for cc in range(CC):
            nc.sync.dma_start(
                out=qTs[:, cc, 0:BLK],
                in_=qT[cc * P : (cc + 1) * P, 0:BLK],
            )
            nc.sync.dma_start(
                out=M8s[:, cc, :], in_=M8[cc * P : (cc + 1) * P, :]
            )
        for g in range(4):
            for cc in range(CC):
                nc.sync.dma_start(
                    out=kTs[:, cc, g * BLK : (g + 1) * BLK],
                    in_=kT[cc * P : (cc + 1) * P, g * BLK : (g + 1) * BLK],
                )
        for mc in range(NN):
            nc.sync.dma_start(
                out=vb8s[:, mc, :], in_=vb8[mc * P : (mc + 1) * P, :]
            )
            nc.sync.dma_start(
                out=vbfs[:, mc, :], in_=vbf[mc * P : (mc + 1) * P, :]
            )
        for g in range(1, 4):
            for cc in range(CC):
                nc.sync.dma_start(
                    out=qTs[:, cc, g * BLK : (g + 1) * BLK],
                    in_=qT[cc * P : (cc + 1) * P, g * BLK : (g + 1) * BLK],
                )

        # ---------------- PE warmup (HAM un-throttle) ----------------
        with (
            tc.tile_pool(name="warm", bufs=1) as warm_pool,
            tc.tile_pool(name="warm_psum", bufs=1, space="PSUM") as warm_psum,
        ):
            wl = warm_pool.tile([P, P], BF16)
            wr = warm_pool.tile([P, BLK], BF16)
            nc.vector.memset(wl, 0.0)
            nc.vector.memset(wr, 0.0)
            wps = warm_psum.tile([P, BLK], F32)
            for i in range(16):
                nc.tensor.matmul(wps, wl, wr, start=True, stop=True)

        t_psum = ctx.enter_context(tc.tile_pool(name="t_psum", bufs=2, space="PSUM"))
        s_psum = ctx.enter_context(tc.tile_pool(name="s_psum", bufs=2, space="PSUM"))

        def tt_block(nb, c2cs=range(CC)):
            # tT[c2, n-block] = M.T-contraction of qT, fp8 DoubleRow over c1
            for c2c in c2cs:
                tp = t_psum.tile([P, BLK], F32, tag="tp", name=f"tp{nb}_{c2c}")
                for j in range(CC // 2):
                    nc.tensor.matmul(
                        tp,
                        M8s[:, 2 * j : 2 * j + 2, c2c * P : (c2c + 1) * P],
                        qTs[:, 2 * j : 2 * j + 2, nb * BLK : (nb + 1) * BLK],
                        start=(j == 0),
                        stop=(j == CC // 2 - 1),
                        perf_mode=DR,
                    )
                nc.vector.tensor_copy(
                    out=tT8[:, c2c, nb * BLK : (nb + 1) * BLK], in_=tp
                )

        tt_block(0)

        # ---------------- steady: S -> exp -> e8 -> PV ----------------
        with (
            tc.tile_pool(name="e8_pool", bufs=2) as e8_pool,
            tc.tile_pool(name="ebf_pool", bufs=4) as ebf_pool,
            tc.tile_pool(name="osb_pool", bufs=2) as osb_pool,
            tc.tile_pool(name="out_pool", bufs=2) as out_pool,
            tc.tile_pool(name="rec_pool", bufs=2) as rec_pool,
            tc.tile_pool(name="o_psum", bufs=2, space="PSUM") as o_psum,
            tc.tile_pool(name="o2_psum", bufs=2, space="PSUM") as o2_psum,
        ):
            for nb in range(NB):
                e8b = e8_pool.tile([P, NN, BLK], F8, tag="e8", name=f"e8_{nb}")
                for mc in range(NN):
                    sp = s_psum.tile([P, BLK], F32, tag="sp", name=f"sp{nb}_{mc}")
                    for j in range(CC // 2):
                        nc.tensor.matmul(
                            sp,
                            kTs[:, 2 * j : 2 * j + 2, mc * P : (mc + 1) * P],
                            tT8[:, 2 * j : 2 * j + 2, nb * BLK : (nb + 1) * BLK],
                            start=(j == 0),
                            stop=(j == CC // 2 - 1),
                            perf_mode=DR,
                        )
                    ebf = ebf_pool.tile([P, BLK], BF16, tag="ebf", name=f"eb{nb}_{mc}")
                    nc.scalar.activation(out=ebf, in_=sp, func=EXP, scale=SCALE)
                    nc.vector.tensor_scalar_add(
                        out=e8b[:, mc, :], in0=ebf, scalar1=-1.0
                    )

                if nb == 0:
                    # colsum(vbf) -> T1b broadcast (ones lhsT replicates the
                    # row-sum to all 128 partitions for free)
                    t1a = o_psum.tile([P, H0], F32, tag="opa", name="t1a")
                    t1c = o2_psum.tile([P, H1], F32, tag="opb", name="t1b")
                    for mc in range(NN):
                        nc.tensor.matmul(
                            t1a, onesW, vbfs[:, mc, 0:H0],
                            start=(mc == 0), stop=(mc == NN - 1),
                        )
                    for mc in range(NN):
                        nc.tensor.matmul(
                            t1c, onesW, vbfs[:, mc, H0 : H0 + H1],
                            start=(mc == 0), stop=(mc == NN - 1),
                        )
                    nc.vector.tensor_copy(out=T1b[:, 0:H0], in_=t1a)
                    nc.vector.tensor_copy(out=T1b[:, H0 : H0 + H1], in_=t1c)

                # tT(nb+1) first half: PE filler while the exp->e8 chain for
                # the last S chunks drains (PV needs the full e8 block)
                if nb + 1 < NB:
                    tt_block(nb + 1, range(0, 3))

                # PV: out*Z[n-sub, 0:770] = e8^T @ vb8 (fp8 DR).
                # Epilogue: Z = 2048 + sum_e (DVE, tiny) -> rec; Scalar scales
                # psum by rec (releasing the banks fast); DVE then folds in the
                # colsum term: o_t = T1b*rec + o_p.  DMA waits only on DVE.
                for ns in range(4):
                    opa = o_psum.tile([P, H0], F32, tag="opa", name=f"oa{nb}_{ns}")
                    opb = o2_psum.tile([P, H1], F32, tag="opb", name=f"ob{nb}_{ns}")
                    for mcp in range(NN // 2):
                        lhs = e8b[:, 2 * mcp : 2 * mcp + 2, ns * P : (ns + 1) * P]
                        nc.tensor.matmul(
                            opa, lhs, vb8s[:, 2 * mcp : 2 * mcp + 2, 0:H0],
                            start=(mcp == 0), stop=(mcp == NN // 2 - 1),
                            perf_mode=DR,
                        )
                    for mcp in range(NN // 2):
                        lhs = e8b[:, 2 * mcp : 2 * mcp + 2, ns * P : (ns + 1) * P]
                        nc.tensor.matmul(
                            opb, lhs, vb8s[:, 2 * mcp : 2 * mcp + 2, H0 : H0 + H1],
                            start=(mcp == 0), stop=(mcp == NN // 2 - 1),
                            perf_mode=DR,
                        )
                    rec = rec_pool.tile([P, 2], F32, tag="rec", name=f"rc{nb}_{ns}")
                    nc.vector.tensor_scalar_add(
                        out=rec[:, 1:2], in0=opb[:, H1 - 2 : H1 - 1], scalar1=2048.0
                    )
                    nc.vector.reciprocal(out=rec[:, 0:1], in_=rec[:, 1:2])
                    o_p = osb_pool.tile([P, C], F32, tag="osb", name=f"os{nb}_{ns}")
                    nc.scalar.activation(
                        out=o_p[:, 0:H0], in_=opa, func=COPY, scale=rec[:, 0:1]
                    )
                    nc.scalar.activation(
                        out=o_p[:, H0:C], in_=opb[:, 0:H0], func=COPY,
                        scale=rec[:, 0:1],
                    )
                    o_t = out_pool.tile([P, C], F32, tag="ot", name=f"ot{nb}_{ns}")
                    nc.vector.scalar_tensor_tensor(
                        out=o_t[:, 0:H0], in0=T1b[:, 0:H0], scalar=rec[:, 0:1],
                        in1=o_p[:, 0:H0],
                        op0=mybir.AluOpType.mult, op1=mybir.AluOpType.add,
                    )
                    nc.vector.scalar_tensor_tensor(
                        out=o_t[:, H0:C], in0=T1b[:, H0:C], scalar=rec[:, 0:1],
                        in1=o_p[:, H0:C],
                        op0=mybir.AluOpType.mult, op1=mybir.AluOpType.add,
                    )
                    row0 = nb * BLK + ns * P
                    nc.sync.dma_start(out=out[row0 : row0 + P, :], in_=o_t)

                    if ns == 0 and nb + 1 < NB:
                        # tT(nb+1) second half
                        tt_block(nb + 1, range(3, CC))

    nc.compile()
    return nc


_NC = None


def _get_nc():
    global _NC
    if _NC is None:
        _NC = build_kernel()
    return _NC


F8NP = mybir.dt.np(F8)
BF16NP = mybir.dt.np(BF16)


def host_prepare(q_x, k_x, v_x, Wq, Wk):
    """Formatting-only host prep: dtype casts, transposes, padding, plus the
    weight-only fold M = Wq.T @ Wk."""
    q_x = np.ascontiguousarray(np.asarray(q_x, dtype=np.float32))
    k_x = np.ascontiguousarray(np.asarray(k_x, dtype=np.float32))
    v_x = np.ascontiguousarray(np.asarray(v_x, dtype=np.float32))
    Wq = np.asarray(Wq, dtype=np.float32)
    Wk = np.asarray(Wk, dtype=np.float32)
    M8np = np.ascontiguousarray((Wq.T @ Wk).astype(F8NP))
    in_maps = []
    for b in range(B):
        qTb = np.ascontiguousarray(q_x[b].astype(F8NP).T)
        kTb = np.ascontiguousarray(k_x[b].astype(F8NP).T)
        vpad = np.zeros((N, VW), np.float32)
        vpad[:, :C] = v_x[b]
        vpad[:, C : C + 2] = 1.0
        in_maps.append(
            {
                "qT": qTb,
                "kT": kTb,
                "M8": M8np,
                "vbf": vpad.astype(BF16NP),
                "vb8": vpad.astype(F8NP),
            }
        )
    return in_maps


def kernel(q_x, k_x, v_x, Wq, Wk):
    from concourse.bass_utils import run_bass_kernel_spmd

    nc = _get_nc()
    in_maps = host_prepare(q_x, k_x, v_x, Wq, Wk)
    res = run_bass_kernel_spmd(nc, in_maps, core_ids=list(range(B)))
    return np.stack([res.results[i]["out"] for i in range(B)], axis=0)
